# revision 12
# baseline (speedup 1.0000x reference)
"""Trainium2 Bass kernel for nn_Attention_84756884619871.

Causal multi-head attention (B=2, S=2048, D=2048, H=16, Dh=128) with RoPE,
fused QKV projection and output projection.

Sharding (8 NeuronCores): data-parallel over batch (2 groups) x
tensor-parallel over heads (4 cores/group, 4 heads each).  Each core:
  - phase A: single pass over x^T chunks computing q^T,k^T (RoPE applied)
    and v for its heads; all matmuls N=512, weights resident in SBUF
  - phase B: flash-style attention in score-transposed space (p^T[t,s]);
    softmax denominator via ones-vector matmul; no max-subtraction
    (scores are small: exp is safe in fp32); per-query-half ctx shards
    AllGather'd over the 4-core group
  - phase C: 512-column slice of the output projection (overlaps the
    second AllGather)
All matmul operands are bf16; host pre-shuffles inputs to [ki, ko, ...]
layouts so every DMA is contiguous per partition.
Host assembles the full [2,2048,2048] output from the 8 column slices.
"""

import numpy as np
import ml_dtypes

import concourse.bass as bass
import concourse.tile as tile
import concourse.mybir as mybir
from concourse import bacc
from contextlib import ExitStack

F32 = mybir.dt.float32
BF16 = mybir.dt.bfloat16
AF = mybir.ActivationFunctionType

D = 2048
S = 2048
NCORES = 8
TPDEG = 4          # tensor-parallel group size (heads)
HLOC = 4           # heads per core
DH = 128
SCALE = float(1.0 / np.sqrt(DH))

_STATE: dict = {}
ABLATE: set = set()   # dev-only: subset of {"A","B","C","AG"} to skip





def _build(krep=1):
    nc = bacc.Bacc("TRN2", target_bir_lowering=False, debug=False, num_devices=NCORES)
    xT = nc.dram_tensor("xT", [128, 4, 16, 512], BF16, kind="ExternalInput")
    wqk = nc.dram_tensor("wqk", [128, 16, 1024], BF16, kind="ExternalInput")
    wv = nc.dram_tensor("wv", [128, 16, 512], BF16, kind="ExternalInput")
    wo = nc.dram_tensor("wo", [128, 16, 512], BF16, kind="ExternalInput")
    cosT = nc.dram_tensor("cosT", [128, S], F32, kind="ExternalInput")
    sinTs = nc.dram_tensor("sinTs", [128, S], F32, kind="ExternalInput")
    trimask = nc.dram_tensor("trimask", [128, 128], BF16, kind="ExternalInput")
    out = nc.dram_tensor("out", [S, 512], F32, kind="ExternalOutput")

    with tile.TileContext(nc) as tc, ExitStack() as top:
        per = top.enter_context(tc.tile_pool(name="persist", bufs=1))
        mask_sb = per.tile([128, 128], BF16, name="mask")
        nc.sync.dma_start(mask_sb[:], trimask.ap())
        ones_c0 = per.tile([128, 1], F32, name="ones_c0")
        nc.vector.memset(ones_c0[:], 1.0)
        ones_col = per.tile([128, 1], BF16, name="ones_col")
        nc.vector.tensor_copy(ones_col[:], ones_c0[:])

        for _ in range(krep):
            _emit_once(nc, tc, xT, wqk, wv, wo, cosT, sinTs, out,
                       mask_sb, ones_col)

    nc.compile()
    return nc


def _emit_once(nc, tc, xT, wqk, wv, wo, cosT, sinTs, out, mask_sb, ones_col):
    with ExitStack() as body:
        dram = body.enter_context(tc.tile_pool(name="dram", bufs=1, space="DRAM"))
        agin = [dram.tile([HLOC * 128, 1024], BF16, name=f"agin{sb}") for sb in range(2)]
        agout = [dram.tile([D, 1024], BF16, name=f"agout{sb}") for sb in range(2)]

        qk_pool = body.enter_context(tc.tile_pool(name="qkpool", bufs=1))
        qrot = [qk_pool.tile([128, S], BF16, name=f"qrot{h}") for h in range(HLOC)]
        krot = [qk_pool.tile([128, S], BF16, name=f"krot{h}") for h in range(HLOC)]
        v_pool = body.enter_context(tc.tile_pool(name="vpool", bufs=1))
        vsb = [v_pool.tile([128, 512], BF16, name=f"v{j}") for j in range(16)]

        # ---- phase A: fused q,k (RoPE) + v projection, one pass over x ----
        if "A" in ABLATE:
            for t in qrot + krot:
                nc.vector.memset(t[:], 0.01)
            for t in vsb:
                nc.vector.memset(t[:], 0.01)
        else:
          with ExitStack() as st:
            w_pool = st.enter_context(tc.tile_pool(name="wp", bufs=1))
            wqk_sb = w_pool.tile([128, 16, 1024], BF16, name="wqk_sb")
            nc.sync.dma_start(wqk_sb[:], wqk.ap())
            wv_sb = w_pool.tile([128, 16, 512], BF16, name="wv_sb")
            nc.sync.dma_start(wv_sb[:], wv.ap())
            cs_pool = st.enter_context(tc.tile_pool(name="csp", bufs=1))
            cos_sb = cs_pool.tile([128, S], F32, name="cos_sb")
            nc.sync.dma_start(cos_sb[:], cosT.ap())
            sin_sb = cs_pool.tile([128, S], F32, name="sin_sb")
            nc.sync.dma_start(sin_sb[:], sinTs.ap())
            xt_pool = st.enter_context(tc.tile_pool(name="xt1", bufs=2))
            tmp_pool = st.enter_context(tc.tile_pool(name="ropetmp", bufs=4))
            ps_qk = st.enter_context(tc.tile_pool(name="psqk", bufs=3, space="PSUM"))
            ps_v = st.enter_context(tc.tile_pool(name="psv", bufs=2, space="PSUM"))
            for sc in range(4):
                xt_c = xt_pool.tile([128, 16, 512], BF16, tag="xt", name=f"xt_{sc}")
                nc.sync.dma_start(xt_c[:], xT.ap()[:, sc])
                sl = slice(512 * sc, 512 * sc + 512)
                for m in range(8):
                    pq = ps_qk.tile([128, 512], F32, tag="psqk", name=f"pq{sc}_{m}")
                    for ko in range(16):
                        nc.tensor.matmul(
                            pq[:], wqk_sb[:, ko, 128 * m:128 * m + 128],
                            xt_c[:, ko, :], start=(ko == 0), stop=(ko == 15),
                        )
                    dest = (qrot[m] if m < 4 else krot[m - 4])[:, sl]
                    t1 = tmp_pool.tile([128, 512], F32, tag="t1", name=f"t1_{sc}_{m}")
                    nc.vector.tensor_mul(t1[:], pq[:], cos_sb[:, sl])
                    t2 = tmp_pool.tile([128, 512], F32, tag="t2", name=f"t2_{sc}_{m}")
                    nc.vector.tensor_mul(t2[0:64, :], pq[64:128, :], sin_sb[0:64, sl])
                    nc.vector.tensor_mul(t2[64:128, :], pq[0:64, :], sin_sb[64:128, sl])
                    nc.vector.tensor_add(dest, t1[:], t2[:])
                for u in range(4):
                    j = 4 * sc + u
                    pv = ps_v.tile([128, 512], F32, tag="psv", name=f"pv{j}")
                    for ko in range(16):
                        nc.tensor.matmul(
                            pv[:], xt_c[:, ko, 128 * u:128 * u + 128],
                            wv_sb[:, ko, :], start=(ko == 0), stop=(ko == 15),
                        )
                    nc.scalar.copy(vsb[j][:], pv[:])

        # ---- phase B + C: attention, AllGather, output projection ---------
        with ExitStack() as st:
            wo_pool = st.enter_context(tc.tile_pool(name="wop", bufs=1))
            wo_sb = wo_pool.tile([128, 16, 512], BF16, name="wo_sb")
            nc.sync.dma_start(wo_sb[:], wo.ap())

            p_pool = st.enter_context(tc.tile_pool(name="pp", bufs=3))
            misc = st.enter_context(tc.tile_pool(name="miscb", bufs=2))
            sc_ps = st.enter_context(tc.tile_pool(name="scps", bufs=3, space="PSUM"))
            ctx_ps = st.enter_context(tc.tile_pool(name="ctxps", bufs=2, space="PSUM"))
            l_ps = st.enter_context(tc.tile_pool(name="lps", bufs=2, space="PSUM"))
            for qt in (range(4) if "B" not in ABLATE else []):
                q0 = 512 * qt
                for h in range(HLOC):
                    ctx = ctx_ps.tile([128, 512], F32, tag="ctx", name=f"ctx{qt}_{h}")
                    lps = l_ps.tile([1, 512], F32, tag="l", name=f"l{qt}_{h}")
                    jmax = 4 * qt + 4
                    for j in range(jmax):
                        dj = j - 4 * qt
                        c0 = max(0, 128 * dj)
                        sc_t = sc_ps.tile([128, 512], F32, tag="scps",
                                          name=f"sc{qt}_{h}_{j}")
                        nc.tensor.matmul(
                            sc_t[:, c0:512], krot[h][:, 128 * j:128 * j + 128],
                            qrot[h][:, q0 + c0:q0 + 512],
                            start=True, stop=True,
                        )
                        p_t = p_pool.tile([128, 512], BF16, tag="p",
                                          name=f"p{qt}_{h}_{j}")
                        nc.scalar.activation(p_t[:, c0:512], sc_t[:, c0:512],
                                             AF.Exp, scale=SCALE)
                        if dj >= 0:
                            dsl = slice(128 * dj, 128 * dj + 128)
                            nc.vector.tensor_mul(p_t[:, dsl], p_t[:, dsl], mask_sb[:])
                        last = (j == jmax - 1)
                        nc.tensor.matmul(
                            ctx[:, c0:512], vsb[j][:, 128 * h:128 * h + 128],
                            p_t[:, c0:512], start=(j == 0), stop=last,
                            skip_group_check=True,
                        )
                        nc.tensor.matmul(
                            lps[0:1, c0:512], ones_col[:], p_t[:, c0:512],
                            start=(j == 0), stop=last, skip_group_check=True,
                        )
                    # normalize: ctxn = ctx * (1/l) broadcast over partitions
                    linv = misc.tile([1, 512], F32, tag="linv", name=f"li{qt}_{h}")
                    nc.vector.reciprocal_approx_fast(out=linv[:], in_=lps[:])
                    bsb = misc.tile([128, 512], F32, tag="bsb", name=f"bs{qt}_{h}")
                    nc.gpsimd.partition_broadcast(bsb[:], linv[0:1, :])
                    ctxn = misc.tile([128, 512], BF16, tag="ctxn", name=f"cn{qt}_{h}")
                    nc.vector.tensor_mul(ctxn[:], ctx[:], bsb[:])
                    sb, qq = qt // 2, 512 * (qt % 2)
                    nc.sync.dma_start(
                        agin[sb][128 * h:128 * h + 128, qq:qq + 512], ctxn[:])
                if qt % 2 == 1 and "AG" not in ABLATE:
                    sb = qt // 2
                    nc.gpsimd.collective_compute(
                        "AllGather", mybir.AluOpType.bypass,
                        ins=[agin[sb][:]], outs=[agout[sb][:]],
                        replica_groups=[[0, 1, 2, 3], [4, 5, 6, 7]],
                    )
            if "B" in ABLATE and "AG" not in ABLATE:
                for sb in range(2):
                    nc.gpsimd.collective_compute(
                        "AllGather", mybir.AluOpType.bypass,
                        ins=[agin[sb][:]], outs=[agout[sb][:]],
                        replica_groups=[[0, 1, 2, 3], [4, 5, 6, 7]],
                    )

            # ---- phase C: output projection (512-col slice, K = all heads)
            if "C" in ABLATE:
                return
            cg_pool = st.enter_context(tc.tile_pool(name="cgp", bufs=1))
            osb_pool = st.enter_context(tc.tile_pool(name="osbp", bufs=3))
            ps_o = st.enter_context(tc.tile_pool(name="pso", bufs=1, space="PSUM"))
            ctxg = []
            for half in range(2):
                cg = cg_pool.tile([128, 16, 1024], BF16, name=f"ctxg{half}")
                if "AG" in ABLATE:
                    src = agin[half][:].rearrange("(ko ki) q -> ki ko q", ki=128)
                    for ko in range(16):
                        nc.sync.dma_start(cg[:, ko, :], src[:, ko % 4, :])
                else:
                    nc.sync.dma_start(
                        cg[:], agout[half][:].rearrange("(ko ki) q -> ki ko q", ki=128))
                ctxg.append(cg)
            for m in range(16):
                half, mm = m // 8, m % 8
                po = ps_o.tile([128, 512], F32, tag="pso", name=f"po{m}")
                for ko in range(16):
                    nc.tensor.matmul(
                        po[:], ctxg[half][:, ko, 128 * mm:128 * mm + 128],
                        wo_sb[:, ko, :], start=(ko == 0), stop=(ko == 15),
                    )
                osb = osb_pool.tile([128, 512], F32, tag="osb", name=f"osb{m}")
                nc.scalar.copy(osb[:], po[:])
                nc.sync.dma_start(out.ap()[128 * m:128 * m + 128, :], osb[:])


def _get_runner(krep=1):
    """Build (once) a persistent jitted SPMD executor for the kernel program."""
    key = ("runner", krep)
    if key in _STATE:
        return _STATE[key]
    import jax
    from jax.sharding import Mesh, PartitionSpec
    from jax.experimental.shard_map import shard_map
    from concourse import bass2jax

    nc = _build(krep)
    bass2jax.install_neuronx_cc_hook()

    in_names, out_names, out_avals = [], [], []
    for alloc in nc.m.functions[0].allocations:
        if not isinstance(alloc, mybir.MemoryLocationSet):
            continue
        name = alloc.memorylocations[0].name
        pname = nc.partition_id_tensor.name if nc.partition_id_tensor else None
        if alloc.kind == "ExternalInput":
            if name != pname:
                in_names.append(name)
        elif alloc.kind == "ExternalOutput":
            out_names.append(name)
            out_avals.append(
                jax.core.ShapedArray(tuple(alloc.tensor_shape),
                                     mybir.dt.np(alloc.dtype))
            )
    n_params = len(in_names)
    all_in = list(in_names) + list(out_names)
    pname = nc.partition_id_tensor.name if nc.partition_id_tensor else None
    if pname is not None:
        all_in.append(pname)

    def _body(*args):
        operands = list(args)
        if pname is not None:
            operands.append(bass2jax.partition_id_tensor())
        outs = bass2jax._bass_exec_p.bind(
            *operands,
            out_avals=tuple(out_avals),
            in_names=tuple(all_in),
            out_names=tuple(out_names),
            lowering_input_output_aliases=(),
            sim_require_finite=False,
            sim_require_nnan=False,
            nc=nc,
        )
        return tuple(outs)

    devices = jax.devices()[:NCORES]
    mesh = Mesh(np.asarray(devices), ("core",))
    specs = (PartitionSpec("core"),)
    sharded = jax.jit(
        shard_map(
            _body, mesh=mesh,
            in_specs=specs * (n_params + len(out_names)),
            out_specs=specs * len(out_names),
            check_rep=False,
        ),
        keep_unused=True,
    )
    runner = {
        "fn": sharded, "in_names": in_names, "out_names": out_names,
        "out_avals": out_avals, "n_params": n_params, "nc": nc,
    }
    _STATE[key] = runner
    return runner


def _kiko(w):
    """[D, C] -> [128, 16, C] with D = ko*128 + ki."""
    Dd, C = w.shape
    return np.ascontiguousarray(w.reshape(16, 128, C).transpose(1, 0, 2))


def _prep_inputs(x, cos, sin, w_qkv, w_o):
    """Host-side sharding: per-core input dict list."""
    x = np.asarray(x, dtype=np.float32)
    cos = np.asarray(cos, dtype=np.float32)
    sin = np.asarray(sin, dtype=np.float32)
    w_qkv = np.asarray(w_qkv, dtype=np.float32)
    w_o = np.asarray(w_o, dtype=np.float32)
    bf = ml_dtypes.bfloat16

    cosT = np.ascontiguousarray(cos.T)                      # [128, S]
    sinT = sin.T
    sinTs = np.ascontiguousarray(
        np.concatenate([-sinT[0:64], sinT[64:128]], axis=0))
    pp, ff = np.meshgrid(np.arange(128), np.arange(128), indexing="ij")
    trimask = (pp <= ff).astype(np.float32)                 # t <= s

    xTs = []
    for b in range(2):
        xb = x[b].T                                         # [D, S]
        # [ki, sc, ko, si] contiguous per partition chunk
        x4 = xb.reshape(16, 128, 4, 512).transpose(1, 2, 0, 3)
        xTs.append(np.ascontiguousarray(x4).astype(bf))

    in_maps = []
    for c in range(NCORES):
        b, tp = c // TPDEG, c % TPDEG
        cs = 512 * tp
        wq = w_qkv[:, cs:cs + 512]
        wk = w_qkv[:, D + cs:D + cs + 512]
        wqk = _kiko(np.concatenate([wq, wk], axis=1)).astype(bf)
        wvs = _kiko(w_qkv[:, 2 * D + cs:2 * D + cs + 512]).astype(bf)
        wos = _kiko(w_o[:, cs:cs + 512]).astype(bf)
        in_maps.append({
            "xT": xTs[b], "wqk": wqk, "wv": wvs, "wo": wos,
            "cosT": cosT, "sinTs": sinTs, "trimask": trimask.astype(bf),
        })
    return in_maps


def _run(in_maps):
    import jax
    r = _get_runner()
    concat = [
        np.concatenate([np.asarray(in_maps[c][n]) for c in range(NCORES)], axis=0)
        for n in r["in_names"]
    ]
    zeros = [
        np.zeros((NCORES * a.shape[0],) + tuple(a.shape[1:]), a.dtype)
        for a in r["out_avals"]
    ]
    outs = r["fn"](*concat, *zeros)
    outs = [np.asarray(o) for o in jax.block_until_ready(outs)]
    per_core = []
    for c in range(NCORES):
        d = {}
        for i, n in enumerate(r["out_names"]):
            shp = r["out_avals"][i].shape
            d[n] = outs[i].reshape((NCORES,) + shp)[c]
        per_core.append(d)
    return per_core


def kernel(x, cos, sin, w_qkv, w_o):
    in_maps = _prep_inputs(x, cos, sin, w_qkv, w_o)
    results = _run(in_maps)
    B = x.shape[0]
    out = np.empty((B, S, D), dtype=np.float32)
    for c in range(NCORES):
        b, tp = c // TPDEG, c % TPDEG
        out[b, :, 512 * tp:512 * tp + 512] = results[c]["out"]
    return out


# revision 35
# speedup vs baseline: 7.4150x; 7.4150x over previous
"""Trainium2 Bass kernel for nn_Attention_84756884619871.

Causal multi-head attention (B=2, S=2048, D=2048, H=16, Dh=128) with RoPE,
fused QKV projection and output projection.

Sharding (8 NeuronCores): data-parallel over batch (2 groups) x
tensor-parallel over heads (4 cores/group, 4 heads each).  Each core:
  - phase A: single pass over x^T chunks computing q^T,k^T (RoPE applied)
    and v for its heads; all matmuls N=512, weights resident in SBUF
  - phase B: flash-style attention in score-transposed space (p^T[t,s]);
    softmax denominator via ones-vector matmul; no max-subtraction
    (scores are small: exp is safe in fp32); per-query-half ctx shards
    AllGather'd over the 4-core group
  - phase C: 512-column slice of the output projection (overlaps the
    second AllGather)
All matmul operands are bf16; host pre-shuffles inputs to [ki, ko, ...]
layouts so every DMA is contiguous per partition.
Host assembles the full [2,2048,2048] output from the 8 column slices.
"""

import numpy as np
import ml_dtypes

import concourse.bass as bass
import concourse.bass_isa as bass_isa
import concourse.tile as tile
from concourse.tile_rust import add_dep_helper
import concourse.mybir as mybir
from concourse import bacc
from contextlib import ExitStack

F32 = mybir.dt.float32
BF16 = mybir.dt.bfloat16
AF = mybir.ActivationFunctionType

D = 2048
S = 2048
NCORES = 8
TPDEG = 4          # tensor-parallel group size (heads)
HLOC = 4           # heads per core
DH = 128
SCALE = float(1.0 / np.sqrt(DH))

_STATE: dict = {}
ABLATE: set = set()   # dev-only: subset of {"A","B","C","AG"} to skip





def _build(krep=1):
    nc = bacc.Bacc("TRN2", target_bir_lowering=False, debug=False, num_devices=NCORES)
    xT = nc.dram_tensor("xT", [128, 4, 16, 512], BF16, kind="ExternalInput")
    wqk = nc.dram_tensor("wqk", [128, 8, 16, 128], BF16, kind="ExternalInput")
    wv = nc.dram_tensor("wv", [128, 16, 512], BF16, kind="ExternalInput")
    wo = nc.dram_tensor("wo", [128, 16, 512], BF16, kind="ExternalInput")
    cosT = nc.dram_tensor("cosT", [128, S], F32, kind="ExternalInput")
    sinTs = nc.dram_tensor("sinTs", [128, S], F32, kind="ExternalInput")
    trimask = nc.dram_tensor("trimask", [128, 128], BF16, kind="ExternalInput")
    out = nc.dram_tensor("out", [S, 512], F32, kind="ExternalOutput")

    with tile.TileContext(nc) as tc, ExitStack() as top:
        per = top.enter_context(tc.tile_pool(name="persist", bufs=1))
        mask_sb = per.tile([128, 128], BF16, name="mask")
        nc.scalar.dma_start(mask_sb[:], trimask.ap())

        for _ in range(krep):
            _emit_once(nc, tc, xT, wqk, wv, wo, cosT, sinTs, out, mask_sb)

    nc.compile()
    return nc


def _emit_once(nc, tc, xT, wqk, wv, wo, cosT, sinTs, out, mask_sb):
    with ExitStack() as body:
        dram = body.enter_context(tc.tile_pool(name="dram", bufs=1, space="DRAM"))
        agin = [dram.tile([HLOC * 128, 512], BF16, name=f"agin{qt}") for qt in range(4)]
        agout = [dram.tile([D, 512], BF16, name=f"agout{qt}") for qt in range(4)]

        qk_pool = body.enter_context(tc.tile_pool(name="qkpool", bufs=1))
        qrot = [qk_pool.tile([128, S], BF16, name=f"qrot{h}") for h in range(HLOC)]
        krot = [qk_pool.tile([128, S], BF16, name=f"krot{h}") for h in range(HLOC)]
        v_pool = body.enter_context(tc.tile_pool(name="vpool", bufs=1))
        vsb = [v_pool.tile([128, 512], BF16, name=f"v{j}") for j in range(16)]

        # ---- phase A: fused q,k (RoPE) + v projection, one pass over x ----
        if "A" in ABLATE:
            for t in qrot + krot:
                nc.vector.memset(t[:], 0.01)
            for t in vsb:
                nc.vector.memset(t[:], 0.01)
        else:
          with ExitStack() as st:
            # DMA ordering: x chunk 0 + cos/sin go on the ACT HWDGE ring
            # (nc.scalar) in parallel with weights on the SP ring (nc.sync),
            # so the first matmul isn't gated on the whole weight burst.
            xt_pool = st.enter_context(tc.tile_pool(name="xt1", bufs=2))
            xt_c0 = xt_pool.tile([128, 16, 512], BF16, tag="xt", name="xt_0")
            nc.scalar.dma_start(xt_c0[:], xT.ap()[:, 0])
            w_pool = st.enter_context(tc.tile_pool(name="wp", bufs=1))
            wqk_m = []
            for m in range(8):
                wm = w_pool.tile([128, 16, 128], BF16, name=f"wqk_m{m}")
                nc.sync.dma_start(wm[:], wqk.ap()[:, m])
                wqk_m.append(wm)
            wv_sb = w_pool.tile([128, 16, 512], BF16, name="wv_sb")
            nc.sync.dma_start(wv_sb[:], wv.ap())
            cs_pool = st.enter_context(tc.tile_pool(name="csp", bufs=1))
            cos_sb = cs_pool.tile([128, S], F32, name="cos_sb")
            sin_sb = cs_pool.tile([128, S], F32, name="sin_sb")
            for g in range(4):
                gs = slice(512 * g, 512 * g + 512)
                nc.scalar.dma_start(cos_sb[:, gs], cosT.ap()[:, gs])
                nc.scalar.dma_start(sin_sb[:, gs], sinTs.ap()[:, gs])
            tmp_pool = st.enter_context(tc.tile_pool(name="ropetmp", bufs=4))
            ps_qk = st.enter_context(tc.tile_pool(name="psqk", bufs=3, space="PSUM"))
            ps_v = st.enter_context(tc.tile_pool(name="psv", bufs=2, space="PSUM"))
            for sc in range(4):
                if sc == 0:
                    xt_c = xt_c0
                else:
                    xt_c = xt_pool.tile([128, 16, 512], BF16, tag="xt",
                                        name=f"xt_{sc}")
                    nc.scalar.dma_start(xt_c[:], xT.ap()[:, sc])
                sl = slice(512 * sc, 512 * sc + 512)
                for m in range(8):
                    pq = ps_qk.tile([128, 512], F32, tag="psqk", name=f"pq{sc}_{m}")
                    for ko in range(16):
                        nc.tensor.matmul(
                            pq[:], wqk_m[m][:, ko, :],
                            xt_c[:, ko, :], start=(ko == 0), stop=(ko == 15),
                        )
                    dest = (qrot[m] if m < 4 else krot[m - 4])[:, sl]
                    t1 = tmp_pool.tile([128, 512], F32, tag="t1", name=f"t1_{sc}_{m}")
                    nc.vector.tensor_mul(t1[:], pq[:], cos_sb[:, sl])
                    t2 = tmp_pool.tile([128, 512], F32, tag="t2", name=f"t2_{sc}_{m}")
                    nc.vector.tensor_mul(t2[0:64, :], pq[64:128, :], sin_sb[0:64, sl])
                    nc.vector.tensor_mul(t2[64:128, :], pq[0:64, :], sin_sb[64:128, sl])
                    nc.vector.tensor_add(dest, t1[:], t2[:])
                for u in range(4):
                    j = 4 * sc + u
                    pv = ps_v.tile([128, 512], F32, tag="psv", name=f"pv{j}")
                    for ko in range(16):
                        nc.tensor.matmul(
                            pv[:], xt_c[:, ko, 128 * u:128 * u + 128],
                            wv_sb[:, ko, :], start=(ko == 0), stop=(ko == 15),
                        )
                    nc.scalar.copy(vsb[j][:], pv[:])

        # ---- phase B + C: attention, AllGather, output projection ---------
        with ExitStack() as st:
            wo_pool = st.enter_context(tc.tile_pool(name="wop", bufs=1))
            wo_sb = wo_pool.tile([128, 16, 512], BF16, name="wo_sb")
            nc.sync.dma_start(wo_sb[:], wo.ap())

            p_pool = st.enter_context(tc.tile_pool(name="pp", bufs=5))
            pacc_pool = st.enter_context(tc.tile_pool(name="paccp", bufs=2))
            misc = st.enter_context(tc.tile_pool(name="miscb", bufs=2))
            sc_ps = st.enter_context(tc.tile_pool(name="scps", bufs=4, space="PSUM"))
            ctx_ps = st.enter_context(tc.tile_pool(name="ctxps", bufs=2, space="PSUM"))
            cg_pool = st.enter_context(tc.tile_pool(name="cgp", bufs=1))
            ctxg = [cg_pool.tile([128, 16, 512], BF16, name=f"ctxg{qt}")
                    for qt in range(4)]
            for qt in (range(4) if "B" not in ABLATE else []):
                q0 = 512 * qt
                for h in range(HLOC):
                    ctx = ctx_ps.tile([128, 512], F32, tag="ctx", name=f"ctx{qt}_{h}")
                    pacc = pacc_pool.tile([128, 512], F32, tag="pacc",
                                          name=f"pa{qt}_{h}")
                    jmax = 4 * qt + 4
                    for j in range(jmax):
                        dj = j - 4 * qt
                        c0 = max(0, 128 * dj)
                        sc_t = sc_ps.tile([128, 512], F32, tag="scps",
                                          name=f"sc{qt}_{h}_{j}")
                        nc.tensor.matmul(
                            sc_t[:, c0:512], krot[h][:, 128 * j:128 * j + 128],
                            qrot[h][:, q0 + c0:q0 + 512],
                            start=True, stop=True,
                        )
                        p_t = p_pool.tile([128, 512], BF16, tag="p",
                                          name=f"p{qt}_{h}_{j}")
                        nc.scalar.activation(p_t[:, c0:512], sc_t[:, c0:512],
                                             AF.Exp, scale=SCALE)
                        if dj >= 0:
                            dsl = slice(128 * dj, 128 * dj + 128)
                            nc.vector.tensor_mul(p_t[:, dsl], p_t[:, dsl], mask_sb[:])
                        last = (j == jmax - 1)
                        last_attn_mm = nc.tensor.matmul(
                            ctx[:, c0:512], vsb[j][:, 128 * h:128 * h + 128],
                            p_t[:, c0:512], start=(j == 0), stop=last,
                            skip_group_check=True,
                        )
                        # softmax denominator: accumulate p on DVE (the
                        # partition-sum happens once per tile on GpSimd)
                        if j == 0:
                            nc.vector.tensor_copy(pacc[:], p_t[:, 0:512])
                        else:
                            nc.vector.tensor_add(pacc[:, c0:512], pacc[:, c0:512],
                                                 p_t[:, c0:512])
                    # normalize: ctxn = ctx * (1/l); l = partition-sum of pacc
                    l_all = misc.tile([128, 512], F32, tag="lall", name=f"la{qt}_{h}")
                    nc.gpsimd.partition_all_reduce(
                        l_all[:], pacc[:], 128, bass_isa.ReduceOp.add)
                    linv = misc.tile([128, 512], F32, tag="linv", name=f"li{qt}_{h}")
                    nc.vector.reciprocal_approx_fast(out=linv[:], in_=l_all[:])
                    ctxn = misc.tile([128, 512], BF16, tag="ctxn", name=f"cn{qt}_{h}")
                    nc.vector.tensor_mul(ctxn[:], ctx[:], linv[:])
                    nc.sync.dma_start(
                        agin[qt][128 * h:128 * h + 128, :], ctxn[:])
                if "AG" not in ABLATE:
                    nc.gpsimd.collective_compute(
                        "AllGather", mybir.AluOpType.bypass,
                        ins=[agin[qt][:]], outs=[agout[qt][:]],
                        replica_groups=[[0, 1, 2, 3], [4, 5, 6, 7]],
                    )
                    nc.scalar.dma_start(
                        ctxg[qt][:],
                        agout[qt][:].rearrange("(ko ki) q -> ki ko q", ki=128))
            if "B" in ABLATE and "AG" not in ABLATE:
                for qt in range(4):
                    nc.gpsimd.collective_compute(
                        "AllGather", mybir.AluOpType.bypass,
                        ins=[agin[qt][:]], outs=[agout[qt][:]],
                        replica_groups=[[0, 1, 2, 3], [4, 5, 6, 7]],
                    )
                    nc.scalar.dma_start(
                        ctxg[qt][:],
                        agout[qt][:].rearrange("(ko ki) q -> ki ko q", ki=128))

            # ---- phase C: output projection (512-col slice, K = all heads)
            if "C" in ABLATE:
                return
            osb_pool = st.enter_context(tc.tile_pool(name="osbp", bufs=3))
            ps_o = st.enter_context(tc.tile_pool(name="pso", bufs=2, space="PSUM"))
            if "AG" in ABLATE:
                for qt in range(4):
                    src = agin[qt][:].rearrange("(ko ki) q -> ki ko q", ki=128)
                    for ko in range(16):
                        nc.sync.dma_start(ctxg[qt][:, ko, :], src[:, ko % 4, :])
            for m in range(16):
                qt, mm = m // 4, m % 4
                po = ps_o.tile([128, 512], F32, tag="pso", name=f"po{m}")
                for ko in range(16):
                    cmm = nc.tensor.matmul(
                        po[:], ctxg[qt][:, ko, 128 * mm:128 * mm + 128],
                        wo_sb[:, ko, :], start=(ko == 0), stop=(ko == 15),
                    )
                    if ko == 0 and "B" not in ABLATE:
                        # Hold phase-C PE work behind attention: otherwise the
                        # scheduler hoists it into transient mid-attention
                        # stalls where it blocks PE waiting on collectives,
                        # and nothing remains to cover the final AG latency.
                        add_dep_helper(cmm.ins, last_attn_mm.ins, sync=False,
                                       reason="phase C after attention")
                osb = osb_pool.tile([128, 512], F32, tag="osb", name=f"osb{m}")
                nc.scalar.copy(osb[:], po[:])
                nc.sync.dma_start(out.ap()[128 * m:128 * m + 128, :], osb[:])


def _get_runner(krep=1):
    """Build (once) a persistent jitted SPMD executor for the kernel program."""
    key = ("runner", krep)
    if key in _STATE:
        return _STATE[key]
    import jax
    from jax.sharding import Mesh, PartitionSpec
    from jax.experimental.shard_map import shard_map
    from concourse import bass2jax

    nc = _build(krep)
    bass2jax.install_neuronx_cc_hook()

    in_names, out_names, out_avals = [], [], []
    for alloc in nc.m.functions[0].allocations:
        if not isinstance(alloc, mybir.MemoryLocationSet):
            continue
        name = alloc.memorylocations[0].name
        pname = nc.partition_id_tensor.name if nc.partition_id_tensor else None
        if alloc.kind == "ExternalInput":
            if name != pname:
                in_names.append(name)
        elif alloc.kind == "ExternalOutput":
            out_names.append(name)
            out_avals.append(
                jax.core.ShapedArray(tuple(alloc.tensor_shape),
                                     mybir.dt.np(alloc.dtype))
            )
    n_params = len(in_names)
    all_in = list(in_names) + list(out_names)
    pname = nc.partition_id_tensor.name if nc.partition_id_tensor else None
    if pname is not None:
        all_in.append(pname)

    def _body(*args):
        operands = list(args)
        if pname is not None:
            operands.append(bass2jax.partition_id_tensor())
        outs = bass2jax._bass_exec_p.bind(
            *operands,
            out_avals=tuple(out_avals),
            in_names=tuple(all_in),
            out_names=tuple(out_names),
            lowering_input_output_aliases=(),
            sim_require_finite=False,
            sim_require_nnan=False,
            nc=nc,
        )
        return tuple(outs)

    devices = jax.devices()[:NCORES]
    mesh = Mesh(np.asarray(devices), ("core",))
    specs = (PartitionSpec("core"),)
    sharded = jax.jit(
        shard_map(
            _body, mesh=mesh,
            in_specs=specs * (n_params + len(out_names)),
            out_specs=specs * len(out_names),
            check_rep=False,
        ),
        keep_unused=True,
    )
    runner = {
        "fn": sharded, "in_names": in_names, "out_names": out_names,
        "out_avals": out_avals, "n_params": n_params, "nc": nc,
    }
    _STATE[key] = runner
    return runner


def _kiko(w):
    """[D, C] -> [128, 16, C] with D = ko*128 + ki."""
    Dd, C = w.shape
    return np.ascontiguousarray(w.reshape(16, 128, C).transpose(1, 0, 2))


def _prep_inputs(x, cos, sin, w_qkv, w_o):
    """Host-side sharding: per-core input dict list."""
    x = np.asarray(x, dtype=np.float32)
    cos = np.asarray(cos, dtype=np.float32)
    sin = np.asarray(sin, dtype=np.float32)
    w_qkv = np.asarray(w_qkv, dtype=np.float32)
    w_o = np.asarray(w_o, dtype=np.float32)
    bf = ml_dtypes.bfloat16

    cosT = np.ascontiguousarray(cos.T)                      # [128, S]
    sinT = sin.T
    sinTs = np.ascontiguousarray(
        np.concatenate([-sinT[0:64], sinT[64:128]], axis=0))
    pp, ff = np.meshgrid(np.arange(128), np.arange(128), indexing="ij")
    trimask = (pp <= ff).astype(np.float32)                 # t <= s

    xTs = []
    for b in range(2):
        xb = x[b].T                                         # [D, S]
        # [ki, sc, ko, si] contiguous per partition chunk
        x4 = xb.reshape(16, 128, 4, 512).transpose(1, 2, 0, 3)
        xTs.append(np.ascontiguousarray(x4).astype(bf))

    in_maps = []
    for c in range(NCORES):
        b, tp = c // TPDEG, c % TPDEG
        cs = 512 * tp
        wq = w_qkv[:, cs:cs + 512]
        wk = w_qkv[:, D + cs:D + cs + 512]
        wqk3 = _kiko(np.concatenate([wq, wk], axis=1))        # [128, 16, 1024]
        wqk = np.ascontiguousarray(
            wqk3.reshape(128, 16, 8, 128).transpose(0, 2, 1, 3)).astype(bf)
        wvs = _kiko(w_qkv[:, 2 * D + cs:2 * D + cs + 512]).astype(bf)
        wos = _kiko(w_o[:, cs:cs + 512]).astype(bf)
        in_maps.append({
            "xT": xTs[b], "wqk": wqk, "wv": wvs, "wo": wos,
            "cosT": cosT, "sinTs": sinTs, "trimask": trimask.astype(bf),
        })
    return in_maps


def _run(in_maps):
    import jax
    r = _get_runner()
    concat = [
        np.concatenate([np.asarray(in_maps[c][n]) for c in range(NCORES)], axis=0)
        for n in r["in_names"]
    ]
    zeros = [
        np.zeros((NCORES * a.shape[0],) + tuple(a.shape[1:]), a.dtype)
        for a in r["out_avals"]
    ]
    outs = r["fn"](*concat, *zeros)
    outs = [np.asarray(o) for o in jax.block_until_ready(outs)]
    per_core = []
    for c in range(NCORES):
        d = {}
        for i, n in enumerate(r["out_names"]):
            shp = r["out_avals"][i].shape
            d[n] = outs[i].reshape((NCORES,) + shp)[c]
        per_core.append(d)
    return per_core


def kernel(x, cos, sin, w_qkv, w_o):
    in_maps = _prep_inputs(x, cos, sin, w_qkv, w_o)
    results = _run(in_maps)
    B = x.shape[0]
    out = np.empty((B, S, D), dtype=np.float32)
    for c in range(NCORES):
        b, tp = c // TPDEG, c % TPDEG
        out[b, :, 512 * tp:512 * tp + 512] = results[c]["out"]
    return out
